# revision 5
# baseline (speedup 1.0000x reference)
"""Trainium2 Bass kernel for nn_CLM_26594437496868 (co-attention + conv/BN/leakyrelu).

Reference computation (b=4, c=64, h=w=64, hw=4096):
  EL = W_lin @ E                       # [c, hw] per sample
  A[n, m] = sum_c EL[c, n] Q[c, m]     # [hw, hw]
  query_c[c, n]    = sum_m Q[c, m] exp(A[n, m]) / sum_m exp(A[n, m])
  exemplar_c[c, n] = sum_m E[c, m] exp(A[m, n]) / sum_m exp(A[m, n])
  out_x = query_c + exemplar_c + E + Q
  y = conv3x3(out_x, W_conv); y = BN(y) * gamma + beta; leaky_relu(y, 0.1)

Sharding: 8 cores = 4 samples x 2 image-halves (rows 0-31 / 32-63).
Each core computes BOTH attention orientations for its 34-row slice
(rows R0-1 .. R0+32, one halo row each side, phantom rows zero-padded
by the host and masked out on device), the conv for all 64 output
channels of its 32 output rows, and local BN partial stats.  One tiny
AllReduce ([64,2] fp32) combines BN stats across all 8 cores.
"""
import sys
if "/opt/trn_rl_repo" not in sys.path:
    sys.path.append("/opt/trn_rl_repo")

import numpy as np

import concourse.bass as bass
import concourse.bacc as bacc
import concourse.tile as tile
import concourse.masks as masks
from concourse import mybir
from concourse import bass_utils

N_CORES = 8
C = 64                    # channels
HW = 4096                 # 64*64
W_IMG = 64
NH = 2176                 # 34 rows * 64 cols  (1 halo row each side)
NOUT = 2048               # 32 output rows * 64
N_BLOCKS = [(0, 512), (512, 512), (1024, 512), (1536, 512), (2048, 128)]
M_CHUNKS = 32             # 4096 / 128
BN_EPS = 1e-5
LEAKY = 0.1

BF16 = mybir.dt.bfloat16
F32 = mybir.dt.float32
NPBF16 = mybir.dt.np(BF16)

_COMPILED = None


def _build_program():
    nc = bacc.Bacc("TRN2", target_bir_lowering=False, debug=False,
                   enable_asserts=True, num_devices=N_CORES)

    # ---- I/O ----
    d_eh = nc.dram_tensor("eh", [C, NH], BF16, kind="ExternalInput").ap()
    d_qh = nc.dram_tensor("qh", [C, NH], BF16, kind="ExternalInput").ap()
    d_xe = nc.dram_tensor("xe", [C, HW], BF16, kind="ExternalInput").ap()
    d_xq = nc.dram_tensor("xq", [C, HW], BF16, kind="ExternalInput").ap()
    d_eqh = nc.dram_tensor("eqh", [C, NH], F32, kind="ExternalInput").ap()
    d_mask = nc.dram_tensor("mask", [C, NH], F32, kind="ExternalInput").ap()
    d_wt = nc.dram_tensor("wt", [C, C], BF16, kind="ExternalInput").ap()
    d_wconv = nc.dram_tensor("wconv", [C, 9, C], BF16, kind="ExternalInput").ap()
    d_gamma = nc.dram_tensor("gamma", [C, 1], F32, kind="ExternalInput").ap()
    d_beta = nc.dram_tensor("beta", [C, 1], F32, kind="ExternalInput").ap()
    d_out = nc.dram_tensor("out", [C, NOUT], F32, kind="ExternalOutput").ap()

    from contextlib import ExitStack
    with tile.TileContext(nc) as tc, ExitStack() as ctx:
        consts = ctx.enter_context(tc.tile_pool(name="consts", bufs=1))
        big = ctx.enter_context(tc.tile_pool(name="big", bufs=1))
        expp = ctx.enter_context(tc.tile_pool(name="expp", bufs=3))
        smalls = ctx.enter_context(tc.tile_pool(name="smalls", bufs=2))
        dram = ctx.enter_context(tc.tile_pool(name="dram", bufs=1, space="DRAM"))
        ps_strip = ctx.enter_context(tc.tile_pool(name="ps_strip", bufs=2, space="PSUM"))
        ps_pv = ctx.enter_context(tc.tile_pool(name="ps_pv", bufs=1, space="PSUM"))
        ps_misc = ctx.enter_context(tc.tile_pool(name="ps_misc", bufs=2, space="PSUM"))

        # ---- load inputs ----
        eh_sb = big.tile([C, NH], BF16)
        qh_sb = big.tile([C, NH], BF16)
        xe_sb = big.tile([C, HW], BF16)
        xq_sb = big.tile([C, HW], BF16)
        eqh_sb = big.tile([C, NH], F32)
        mask_sb = big.tile([C, NH], F32)
        wt_sb = consts.tile([C, C], BF16)
        wconv_sb = consts.tile([C, 9, C], BF16)
        gamma_sb = consts.tile([C, 1], F32)
        beta_sb = consts.tile([C, 1], F32)
        nc.sync.dma_start(out=eh_sb[:], in_=d_eh[:])
        nc.sync.dma_start(out=qh_sb[:], in_=d_qh[:])
        nc.sync.dma_start(out=xe_sb[:], in_=d_xe[:])
        nc.sync.dma_start(out=xq_sb[:], in_=d_xq[:])
        nc.sync.dma_start(out=eqh_sb[:], in_=d_eqh[:])
        nc.sync.dma_start(out=mask_sb[:], in_=d_mask[:])
        nc.sync.dma_start(out=wt_sb[:], in_=d_wt[:])
        nc.sync.dma_start(out=wconv_sb[:], in_=d_wconv[:])
        nc.sync.dma_start(out=gamma_sb[:], in_=d_gamma[:])
        nc.sync.dma_start(out=beta_sb[:], in_=d_beta[:])

        ident = consts.tile([C, C], BF16)
        masks.make_identity(nc, ident[:])
        # ones row at partition 64 (K-row base for the broadcast outer product)
        ones_sb = consts.tile([128, C], BF16)
        nc.vector.memset(ones_sb[64:65, :], 1.0)

        # ---- EL = W_lin @ E  (lhsT = W_lin^T passed as `wt`) ----
        elf_sb = big.tile([C, HW], BF16)        # full-m EL for orientation 2 lhsT
        elh_sb = big.tile([C, NH], BF16)        # half-n EL for orientation 1 rhs
        for j in range(HW // 512):
            ps_el = ps_misc.tile([C, 512], F32, tag="misc")
            nc.tensor.matmul(ps_el[:], wt_sb[:], xe_sb[:, j * 512:(j + 1) * 512],
                             start=True, stop=True)
            nc.vector.tensor_copy(elf_sb[:, j * 512:(j + 1) * 512], ps_el[:])
        for (off, nb) in N_BLOCKS:
            ps_el = ps_misc.tile([C, 512], F32, tag="misc")
            nc.tensor.matmul(ps_el[:, 0:nb], wt_sb[:], eh_sb[:, off:off + nb],
                             start=True, stop=True)
            nc.vector.tensor_copy(elh_sb[:, off:off + nb], ps_el[:, 0:nb])

        # ---- transposed values with ones column: [Q^T | 1], [E^T | 1] ----
        qt_sb = big.tile([128, M_CHUNKS, C + 1], BF16)
        et_sb = big.tile([128, M_CHUNKS, C + 1], BF16)
        nc.vector.memset(qt_sb[:, :, C:C + 1], 1.0)
        nc.vector.memset(et_sb[:, :, C:C + 1], 1.0)
        for j in range(M_CHUNKS):
            pst = ps_misc.tile([128, C], BF16, tag="misc")
            nc.tensor.transpose(pst[:], xq_sb[:, j * 128:(j + 1) * 128], ident[:])
            nc.vector.tensor_copy(qt_sb[:, j, 0:C], pst[:])
            pst2 = ps_misc.tile([128, C], BF16, tag="misc")
            nc.tensor.transpose(pst2[:], xe_sb[:, j * 128:(j + 1) * 128], ident[:])
            nc.vector.tensor_copy(et_sb[:, j, 0:C], pst2[:])

        # ---- conv input (built incrementally): [64, 34 rows, 66 cols] ----
        xpad = big.tile([C, 34, 66], BF16)
        nc.vector.memset(xpad[:], 0.0)

        # ---- attention: both orientations, streamed over m in 2-chunk strips ----
        s_lhs = (xq_sb, elf_sb)       # T1[m,l] = sum_c Q[c,m] ELh[c,l] ; T2 = sum_c EL[c,m] qh[c,l]
        s_rhs = (elh_sb, qh_sb)
        pv_lhs = (qt_sb, et_sb)
        for (off, nb) in N_BLOCKS:
            nrows = nb // W_IMG
            r0 = off // W_IMG
            pvs = []
            for o in (0, 1):
                pv = ps_pv.tile([C + 1, 512], F32, tag=f"pv{o}")
                pvs.append(pv)
                for t in range(M_CHUNKS // 2):
                    sp = ps_strip.tile([128, 2, 512], F32, tag="sp")
                    nc.tensor.matmul(sp[:, 0, 0:nb],
                                     s_lhs[o][:, 256 * t:256 * t + 128],
                                     s_rhs[o][:, off:off + nb],
                                     start=True, stop=True)
                    nc.tensor.matmul(sp[:, 1, 0:nb],
                                     s_lhs[o][:, 256 * t + 128:256 * t + 256],
                                     s_rhs[o][:, off:off + nb],
                                     start=True, stop=True)
                    ex = expp.tile([128, 2, 512], BF16, tag="ex")
                    nc.scalar.activation(out=ex[:, :, 0:nb], in_=sp[:, :, 0:nb],
                                         func=mybir.ActivationFunctionType.Exp)
                    nc.tensor.matmul(pv[:, 0:nb], pv_lhs[o][:, 2 * t, :],
                                     ex[:, 0, 0:nb],
                                     start=(t == 0), stop=False)
                    nc.tensor.matmul(pv[:, 0:nb], pv_lhs[o][:, 2 * t + 1, :],
                                     ex[:, 1, 0:nb],
                                     start=False, stop=(t == M_CHUNKS // 2 - 1))
            # normalize:  O[c, l] / D[l]  via PE broadcast of 1/D
            zs = []
            for o in (0, 1):
                rd = smalls.tile([128, 512], BF16, tag="rd")
                with nc.allow_low_precision(reason="1/D broadcast via bf16 PE outer product"):
                    nc.vector.reciprocal(rd[64:65, 0:nb], pvs[o][C:C + 1, 0:nb])
                bc_ps = ps_misc.tile([C, 512], F32, tag="misc")
                nc.tensor.matmul(bc_ps[:, 0:nb], ones_sb[64:65, :],
                                 rd[64:65, 0:nb], start=True, stop=True)
                bc = smalls.tile([C, 512], F32, tag="bc")
                nc.vector.tensor_copy(bc[:, 0:nb], bc_ps[:, 0:nb])
                z = smalls.tile([C, 512], F32, tag=f"z{o}")
                nc.vector.tensor_mul(z[:, 0:nb], pvs[o][0:C, 0:nb], bc[:, 0:nb])
                zs.append(z)
            zsum = smalls.tile([C, 512], F32, tag="zsum")
            nc.vector.tensor_add(zsum[:, 0:nb], zs[0][:, 0:nb], zs[1][:, 0:nb])
            nc.vector.tensor_add(zsum[:, 0:nb], zsum[:, 0:nb],
                                 eqh_sb[:, off:off + nb])
            # masked cast into the padded conv input
            nc.vector.scalar_tensor_tensor(
                out=xpad[:, r0:r0 + nrows, 1:65],
                in0=zsum[:, 0:nb].rearrange("p (r w) -> p r w", w=W_IMG),
                scalar=1.0,
                in1=mask_sb[:, off:off + nb].rearrange("p (r w) -> p r w", w=W_IMG),
                op0=mybir.AluOpType.mult,
                op1=mybir.AluOpType.mult,
            )

        # ---- conv 3x3 (all 64 out channels, 32 output rows) + BN stats ----
        y_sb = big.tile([C, NOUT], F32)
        st = smalls.tile([C, 4, 6], F32, tag="st")
        for rb in range(4):
            yp = ps_misc.tile([C, 512], F32, tag="misc")
            for tap in range(9):
                dy, dx = tap // 3, tap % 3
                nc.tensor.matmul(
                    yp[:],
                    wconv_sb[:, tap, :],
                    xpad[:, 8 * rb + dy:8 * rb + dy + 8, dx:dx + 64],
                    start=(tap == 0), stop=(tap == 8))
            nc.vector.tensor_copy(y_sb[:, rb * 512:(rb + 1) * 512], yp[:])
            nc.vector.bn_stats(out=st[:, rb, :],
                               in_=y_sb[:, rb * 512:(rb + 1) * 512])
        mv = smalls.tile([C, 2], F32, tag="mv")
        nc.vector.bn_aggr(out=mv[:], in_=st[:])

        # ---- BN stats AllReduce: payload (mean_i, var_i + mean_i^2) ----
        ccs = smalls.tile([C, 2], F32, tag="ccs")
        m2 = smalls.tile([C, 1], F32, tag="m2")
        nc.vector.tensor_mul(m2[:], mv[:, 0:1], mv[:, 0:1])
        nc.vector.tensor_copy(ccs[:, 0:1], mv[:, 0:1])
        nc.vector.tensor_add(ccs[:, 1:2], mv[:, 1:2], m2[:])
        cc_in = dram.tile([C, 2], F32)
        cc_out = dram.tile([C, 2], F32, addr_space="Shared")
        nc.sync.dma_start(out=cc_in[:], in_=ccs[:])
        nc.gpsimd.collective_compute(
            "AllReduce", mybir.AluOpType.add,
            replica_groups=[list(range(N_CORES))],
            ins=[cc_in.opt()], outs=[cc_out.opt()])
        red = smalls.tile([C, 2], F32, tag="red")
        nc.sync.dma_start(out=red[:], in_=cc_out[:])

        # mu = red0/8 ; var = red1/8 - mu^2 ; rstd = exp(-0.5*ln(var+eps))
        mu = smalls.tile([C, 1], F32, tag="mu")
        var = smalls.tile([C, 1], F32, tag="var")
        nc.vector.tensor_scalar_mul(mu[:], red[:, 0:1], 1.0 / N_CORES)
        nc.vector.tensor_scalar_mul(var[:], red[:, 1:2], 1.0 / N_CORES)
        mu2 = smalls.tile([C, 1], F32, tag="mu2")
        nc.vector.tensor_mul(mu2[:], mu[:], mu[:])
        nc.vector.tensor_sub(var[:], var[:], mu2[:])
        nc.vector.tensor_scalar_add(var[:], var[:], BN_EPS)
        lnv = smalls.tile([C, 1], F32, tag="lnv")
        nc.scalar.activation(out=lnv[:], in_=var[:],
                             func=mybir.ActivationFunctionType.Ln)
        rstd = smalls.tile([C, 1], F32, tag="rstd")
        nc.scalar.activation(out=rstd[:], in_=lnv[:],
                             func=mybir.ActivationFunctionType.Exp, scale=-0.5)
        scale_f = smalls.tile([C, 1], F32, tag="scale_f")
        bias_f = smalls.tile([C, 1], F32, tag="bias_f")
        nc.vector.tensor_mul(scale_f[:], gamma_sb[:], rstd[:])
        nc.vector.tensor_mul(bias_f[:], mu[:], scale_f[:])
        nc.vector.tensor_sub(bias_f[:], beta_sb[:], bias_f[:])

        # ---- apply BN + leaky relu, write out ----
        # (ACT Lrelu ignores the alpha operand — hardware uses slope 0.01 —
        #  so build leaky_relu(x) = max(x, 0.1*x) on the vector engine.)
        for rb in range(4):
            lin = smalls.tile([C, 512], F32, tag="lin")
            nc.vector.tensor_scalar(
                out=lin[:], in0=y_sb[:, rb * 512:(rb + 1) * 512],
                scalar1=scale_f[:], scalar2=bias_f[:],
                op0=mybir.AluOpType.mult, op1=mybir.AluOpType.add)
            lin01 = smalls.tile([C, 512], F32, tag="lin01")
            nc.vector.tensor_scalar_mul(lin01[:], lin[:], LEAKY)
            osb = smalls.tile([C, 512], F32, tag="osb")
            nc.vector.tensor_max(osb[:], lin[:], lin01[:])
            nc.sync.dma_start(out=d_out[:, rb * 512:(rb + 1) * 512], in_=osb[:])

    nc.compile()
    return nc


def _get_program():
    global _COMPILED
    if _COMPILED is None:
        _COMPILED = _build_program()
    return _COMPILED


def _make_in_maps(exemplar, query, W_lin, W_conv, gamma, beta):
    E = np.asarray(exemplar, dtype=np.float32).reshape(4, C, HW)
    Q = np.asarray(query, dtype=np.float32).reshape(4, C, HW)
    wt = np.ascontiguousarray(np.asarray(W_lin, np.float32).T).astype(NPBF16)
    wconv = np.ascontiguousarray(
        np.asarray(W_conv, np.float32).transpose(1, 2, 3, 0).reshape(C, 9, C)
    ).astype(NPBF16)
    g = np.asarray(gamma, np.float32).reshape(C, 1)
    b = np.asarray(beta, np.float32).reshape(C, 1)

    zeros = np.zeros((C, W_IMG), np.float32)
    in_maps = []
    for k in range(N_CORES):
        s, h = divmod(k, 2)
        if h == 0:
            sl = lambda X: np.concatenate([zeros, X[s][:, :NH - W_IMG]], axis=1)
        else:
            sl = lambda X: np.concatenate([X[s][:, HW - (NH - W_IMG):], zeros], axis=1)
        eh = sl(E)
        qh = sl(Q)
        mask = np.ones((C, NH), np.float32)
        if h == 0:
            mask[:, :W_IMG] = 0.0
        else:
            mask[:, NH - W_IMG:] = 0.0
        in_maps.append({
            "eh": eh.astype(NPBF16),
            "qh": qh.astype(NPBF16),
            "xe": E[s].astype(NPBF16),
            "xq": Q[s].astype(NPBF16),
            "eqh": (eh + qh),
            "mask": mask,
            "wt": wt,
            "wconv": wconv,
            "gamma": g,
            "beta": b,
        })
    return in_maps


def kernel(exemplar, query, W_lin, W_conv, gamma, beta):
    nc = _get_program()
    in_maps = _make_in_maps(exemplar, query, W_lin, W_conv, gamma, beta)
    res = bass_utils.run_bass_kernel_spmd(
        nc, in_maps, core_ids=list(range(N_CORES)), trace=False)
    out = np.empty((4, C, 64, 64), np.float32)
    for k in range(N_CORES):
        s, h = divmod(k, 2)
        out[s, :, 32 * h:32 * h + 32, :] = \
            res.results[k]["out"].reshape(C, 32, 64)
    return out
